# revision 1
# baseline (speedup 1.0000x reference)
"""Trainium2 Bass kernel for nn_MultiLatentAttention (B=8, S=4096, D=2048, H=16, hd=128, L=16).

Strategy (data-parallel over batch, one batch element per core, plus TP on the
tiny output projections with 3 small collectives):

The reference computes k = LN(x)@Wk, v = LN(x)@Wv (two 32768x2048x2048 GEMMs),
latent cross-attention, and a rank-1 residual broadcast. We restructure so the
giant projections never happen:

  scores[t, hl] = qhat[:,hl] . xtilde[t,:] - c[hl]*mutilde[t]   (contract D)
  where xtilde = x * rsqrt(var+eps) (per-token), qhat = (q @ Wk_head^T) * g,
  c = sum_d qhat, mutilde = mu * rsqrt(var+eps). The LN bias and k-bias cancel
  in softmax exactly. e = exp(scores/sqrt(hd)) unnormalized;
  Z = sum_t e, r = e @ mutilde, u = e^T.T @ xtilde;
  M = (u - r 1^T)/Z; per-head means of M go through Wv/Wlv/Wout (folded with
  ln_g and all biases host-side) to the rank-1 output row; residual-add at end.

All weight-derived small constants (qhat, c, folded biases, weight slices) are
precomputed host-side (pure weight folding, no x involved).
"""

import sys
import functools
import numpy as np
import ml_dtypes

sys.path.insert(0, "/opt/trn_rl_repo")

import concourse.bass as bass
import concourse.mybir as mybir
import concourse.tile as tile
from concourse import bacc
from concourse.bass_utils import run_bass_kernel_spmd

BF = mybir.dt.bfloat16
F32 = mybir.dt.float32
AF = mybir.ActivationFunctionType

P = 128
D = 2048
KT = D // P          # 16 d-tiles
H = 16
HD = 128
L = 16
HL = H * L           # 256 score rows (h-major: hl = h*16 + l)
EPS = 1e-5
INV_SQRT_HD = 1.0 / float(np.sqrt(HD))


def _build(n_cores: int, S: int):
    NB = n_cores
    HPC = H // NB            # heads per core
    SL = D // NB             # d_out slice width per core
    SLT = SL // P            # row-tiles in slice
    NT = S // P              # token tiles
    NQ = 4                   # sweeps (SBUF residency quarters)
    TPQ = NT // NQ           # token tiles per quarter
    assert NT % NQ == 0 and H % NB == 0 and D % NB == 0 and SL % P == 0

    nc = bacc.Bacc(None, target_bir_lowering=False, num_devices=NB)
    groups = [list(range(NB))]

    with tile.TileContext(nc) as tc:
        with tc.tile_pool(name="dram", bufs=1, space="DRAM") as dram:
            def din(name, shape, dt):
                return dram.tile(shape, dt, kind="ExternalInput", name=name, uniquify=False)

            x_d = din("x", [S, D], F32)
            qhatT_d = din("qhatT", [P, KT, HL], BF)
            cneg_d = din("cneg", [1, HL], BF)
            selmat_d = din("selmat", [P, 2, H], F32)
            wv_d = din("wv_s", [P, KT, HPC * P], F32)
            bv_d = din("bv_row", [1, HPC * P], F32)
            wlv_d = din("wlv_r", [P, SLT, D], F32)
            wout_d = din("wout_s", [P, SLT, D], F32)
            biasf_d = din("biasf", [1, D], F32)
            y_d = dram.tile([S, D], F32, kind="ExternalOutput", name="y", uniquify=False)

            # collective bounce buffers
            m_bounce = dram.tile([H, D], F32, name="m_bounce")
            m2_bounce = dram.tile([H, D], F32, name="m2_bounce")
            pp_bounce = dram.tile([D, NB], F32, name="pp_bounce")
            ppr_bounce = dram.tile([SL, NB], F32, name="ppr_bounce")
            op_bounce = dram.tile([NB, D], F32, name="op_bounce")
            ob_bounce = dram.tile([1, D], F32, name="ob_bounce")

            with (
                tc.tile_pool(name="consts", bufs=1) as consts,
                tc.tile_pool(name="resident", bufs=1) as res,
                tc.tile_pool(name="xq", bufs=1) as xq_pool,
            ):
                # ---- load small constants ----
                qhatT = consts.tile([P, KT, HL], BF)
                nc.sync.dma_start(qhatT[:], qhatT_d[:])
                cneg = consts.tile([1, HL], BF)
                nc.sync.dma_start(cneg[:], cneg_d[:])
                selmat = consts.tile([P, 2, H], F32)
                nc.sync.dma_start(selmat[:], selmat_d[:])
                wv_s = consts.tile([P, KT, HPC * P], F32)
                nc.sync.dma_start(wv_s[:], wv_d[:])
                bv_row = consts.tile([1, HPC * P], F32)
                nc.sync.dma_start(bv_row[:], bv_d[:])
                wlv_r = consts.tile([P, SLT, D], F32)
                nc.sync.dma_start(wlv_r[:], wlv_d[:])
                wout_s = consts.tile([P, SLT, D], F32)
                nc.sync.dma_start(wout_s[:], wout_d[:])
                biasf = consts.tile([1, D], F32)
                nc.sync.dma_start(biasf[:], biasf_d[:])

                ident_bf = consts.tile([P, P], BF)
                from concourse.masks import make_identity
                make_identity(nc, ident_bf)
                ident_f = consts.tile([P, P], F32)
                make_identity(nc, ident_f)
                onescol_bf = consts.tile([P, 1], BF)
                nc.any.memset(onescol_bf[:], 1.0)
                ones1_bf = consts.tile([1, NB], BF)
                nc.any.memset(ones1_bf[:], 1.0)
                ones1_f = consts.tile([1, NB], F32)
                nc.any.memset(ones1_f[:], 1.0)
                onescol_f = consts.tile([1, P], F32)
                nc.any.memset(onescol_f[:], 1.0)
                eps_col = consts.tile([P, 1], F32)
                nc.any.memset(eps_col[:], EPS)

                # ---- persistent accumulators ----
                u_acc = res.tile([P, 2, D], F32)
                z_acc = res.tile([P, 2, NQ], F32)     # Z partials per quarter
                r_acc = res.tile([P, 2, NQ], F32)     # r partials per quarter
                mutcols = res.tile([P, NT], BF)       # mutilde per token (column form)

                # ================= PASS 1 =================
                for q in range(NQ):
                    with (
                        tc.tile_pool(name=f"xth{q}", bufs=1) as xth_pool,
                        tc.tile_pool(name=f"eth{q}", bufs=1) as eth_pool,
                        tc.tile_pool(name=f"p1s{q}", bufs=1) as sb,
                    ):
                        xth = xth_pool.tile([P, TPQ, D], BF)       # xtilde quarter
                        eth = eth_pool.tile([P, TPQ, HL], BF)      # e (scoresT) quarter
                        ps_ctx = tc.tile_pool(name=f"p1ps{q}", bufs=2, space="PSUM")
                        ps = ps_ctx.__enter__()
                        ps_zr_ctx = tc.tile_pool(name=f"p1pzr{q}", bufs=1, space="PSUM")
                        ps_zr = ps_zr_ctx.__enter__()
                        # one PSUM bank per accumulation group (HW start=True
                        # clears the whole bank row, so groups must not share)
                        zr_tiles = [ps_zr.tile([P, 1], F32, tag=f"zr{j}", name=f"zr{j}_{q}")
                                    for j in range(4)]

                        for lt in range(TPQ):
                            ti = q * TPQ + lt
                            # stream x tile
                            xf = sb.tile([P, D], F32, tag="xf", bufs=4)
                            nc.sync.dma_start(xf[:], x_d[ti * P:(ti + 1) * P, :])
                            # stats
                            bns = sb.tile([P, 4, 6], F32, tag="bns", bufs=2)
                            for a in range(4):
                                nc.vector.bn_stats(bns[:, a, :], xf[:, a * 512:(a + 1) * 512])
                            mv = sb.tile([P, 2], F32, tag="mv", bufs=2)
                            nc.vector.bn_aggr(mv[:], bns[:])
                            sig = sb.tile([P, 1], F32, tag="sig", bufs=2)
                            nc.scalar.activation(sig[:], mv[:, 1:2], AF.Sqrt, bias=eps_col[:])
                            alpha = sb.tile([P, 1], F32, tag="alpha", bufs=2)
                            nc.vector.reciprocal(alpha[:], sig[:])
                            mut = sb.tile([P, 1], F32, tag="mut", bufs=2)
                            nc.vector.tensor_tensor(mut[:], mv[:, 0:1], alpha[:], mybir.AluOpType.mult)
                            nc.vector.tensor_copy(out=mutcols[:, ti:ti + 1], in_=mut[:])
                            # xtilde (scaled cast) into resident quarter buffer
                            nc.scalar.activation(xth[:, lt, :], xf[:], AF.Copy, scale=alpha[:])
                            # transpose xtilde tile -> [d, tok] tiles
                            xtT = sb.tile([P, KT, P], BF, tag="xtT", bufs=3)
                            nc.sync.dma_start_transpose(xtT[:], xth[:, lt, :])
                            # mutilde row via PE transpose
                            mur_ps = ps.tile([1, P], F32, tag="murp", bufs=1)
                            nc.tensor.matmul(mur_ps[:], mutcols[:, ti:ti + 1], ident_bf[:],
                                             start=True, stop=True)
                            murow = sb.tile([1, P], BF, tag="murow", bufs=2)
                            nc.scalar.copy(out=murow[:], in_=mur_ps[:])
                            # scoresT[t, hl] accumulation
                            sc_ps = ps.tile([P, HL], F32, tag="scps", bufs=3)
                            for kt in range(KT):
                                nc.tensor.matmul(sc_ps[:], xtT[:, kt, :], qhatT[:, kt, :],
                                                 start=(kt == 0), stop=False)
                            nc.tensor.matmul(sc_ps[:], murow[:], cneg[:], start=False, stop=True)
                            # e = exp(scores/sqrt(hd))
                            nc.scalar.activation(eth[:, lt, :], sc_ps[:], AF.Exp,
                                                 scale=INV_SQRT_HD)
                            # Z, r accumulation (columns of psum_zr)
                            for mh in range(2):
                                nc.tensor.matmul(zr_tiles[mh][:],
                                                 eth[:, lt, mh * P:(mh + 1) * P],
                                                 onescol_bf[:],
                                                 start=(lt == 0), stop=(lt == TPQ - 1),
                                                 skip_group_check=True)
                                nc.tensor.matmul(zr_tiles[2 + mh][:],
                                                 eth[:, lt, mh * P:(mh + 1) * P],
                                                 mutcols[:, ti:ti + 1],
                                                 start=(lt == 0), stop=(lt == TPQ - 1),
                                                 skip_group_check=True)
                        # spill Z/r
                        nc.scalar.copy(out=z_acc[:, 0, q:q + 1], in_=zr_tiles[0][:])
                        nc.scalar.copy(out=z_acc[:, 1, q:q + 1], in_=zr_tiles[1][:])
                        nc.scalar.copy(out=r_acc[:, 0, q:q + 1], in_=zr_tiles[2][:])
                        nc.scalar.copy(out=r_acc[:, 1, q:q + 1], in_=zr_tiles[3][:])
                        ps_zr_ctx.__exit__(None, None, None)
                        ps_ctx.__exit__(None, None, None)

                        # ---- u sweep for this quarter ----
                        with tc.tile_pool(name=f"ups{q}", bufs=1, space="PSUM") as ups:
                            for mh in range(2):
                                psum_u = ups.tile([P, D], F32, tag="upsum", bufs=1)
                                for kt in range(TPQ):
                                    for nch in range(D // 512):
                                        nc.tensor.matmul(
                                            psum_u[:, nch * 512:(nch + 1) * 512],
                                            eth[:, kt, mh * P:(mh + 1) * P],
                                            xth[:, kt, nch * 512:(nch + 1) * 512],
                                            start=(kt == 0), stop=(kt == TPQ - 1),
                                            skip_group_check=True)
                                if q == 0:
                                    nc.scalar.copy(out=u_acc[:, mh, :], in_=psum_u[:])
                                else:
                                    nc.vector.tensor_tensor(u_acc[:, mh, :], u_acc[:, mh, :],
                                                            psum_u[:], mybir.AluOpType.add)

                # ================= STAGE C =================
                NPF = 4
                pf_pool_ctx = tc.tile_pool(name="pf", bufs=1)
                pf_pool = pf_pool_ctx.__enter__()
                pf = pf_pool.tile([P, NPF, D], F32)
                for ti in range(NPF):
                    nc.sync.dma_start(pf[:, ti, :], x_d[ti * P:(ti + 1) * P, :])
                with tc.tile_pool(name="c_sb", bufs=1) as csb:
                    # Z, r totals and reciprocal
                    zt = csb.tile([P, 2], F32)
                    nc.vector.tensor_reduce(zt[:], z_acc[:], mybir.AxisListType.X,
                                            mybir.AluOpType.add)
                    rt = csb.tile([P, 2], F32)
                    nc.vector.tensor_reduce(rt[:], r_acc[:], mybir.AxisListType.X,
                                            mybir.AluOpType.add)
                    rz = csb.tile([P, 2], F32)
                    nc.vector.reciprocal(rz[:], zt[:])
                    # M' = (u - r)/Z  (bf16)
                    mp = csb.tile([P, 2, D], F32)
                    for mh in range(2):
                        nc.vector.tensor_scalar(mp[:, mh, :], u_acc[:, mh, :],
                                                rt[:, mh:mh + 1], rz[:, mh:mh + 1],
                                                mybir.AluOpType.subtract,
                                                mybir.AluOpType.mult)
                    # mbar = per-head means [H, D]
                    mb_sb = csb.tile([H, D], F32)
                    with tc.tile_pool(name="c_ps_mb", bufs=1, space="PSUM") as cps0:
                        mb_ps = cps0.tile([H, D], F32)
                        for mh in range(2):
                            for nch in range(D // 512):
                                nc.tensor.matmul(mb_ps[:, nch * 512:(nch + 1) * 512],
                                                 selmat[:, mh, :],
                                                 mp[:, mh, nch * 512:(nch + 1) * 512],
                                                 start=(mh == 0), stop=(mh == 1),
                                                 skip_group_check=True)
                        nc.scalar.copy(out=mb_sb[:], in_=mb_ps[:])
                    nc.sync.dma_start(m_bounce[:], mb_sb[:])
                    nc.gpsimd.collective_compute(
                        "AllToAll", mybir.AluOpType.bypass, replica_groups=groups,
                        ins=[m_bounce[:].opt()], outs=[m2_bounce[:].opt()])
                    # load [d, (kt, hh, b)] tiles of gathered mbar
                    mT = csb.tile([P, KT, HPC, NB], F32)
                    m2_sb = csb.tile([H, D], F32)
                    nc.sync.dma_start(m2_sb[:], m2_bounce[:])
                    with tc.tile_pool(name="c_ps_mt", bufs=1, space="PSUM") as cpsm:
                        for kt in range(KT):
                            mt_ps = cpsm.tile([P, H], F32, tag="mtps", bufs=2)
                            nc.tensor.matmul(mt_ps[:], m2_sb[:, kt * P:(kt + 1) * P],
                                             ident_f[:H, :H], start=True, stop=True)
                            nc.scalar.copy(
                                out=mT[:, kt, :, :].rearrange("p h b -> p b h"),
                                in_=mt_ps[:])
                    # cbarT slice: per local head: psum [NB, P] -> transpose -> [P, NB]
                    cT_loc = csb.tile([P, HPC, NB], F32)
                    with tc.tile_pool(name="c_ps_cb", bufs=1, space="PSUM") as cps1:
                        for hh in range(HPC):
                            cb_ps = cps1.tile([NB, P], F32, tag="cbps", bufs=2)
                            for kt in range(KT):
                                nc.tensor.matmul(cb_ps[:], mT[:, kt, hh, :],
                                                 wv_s[:, kt, hh * P:(hh + 1) * P],
                                                 start=(kt == 0), stop=False)
                            nc.tensor.matmul(cb_ps[:], ones1_f[:],
                                             bv_row[:, hh * P:(hh + 1) * P],
                                             start=False, stop=True)
                            cb_sb = csb.tile([NB, P], F32, tag="cbsb", bufs=2)
                            nc.scalar.copy(out=cb_sb[:], in_=cb_ps[:])
                            ct_ps = cps1.tile([P, NB], F32, tag="ctps", bufs=2)
                            nc.tensor.matmul(ct_ps[:], cb_sb[:], ident_f[:NB, :NB],
                                             start=True, stop=True)
                            nc.scalar.copy(out=cT_loc[:, hh, :], in_=ct_ps[:])
                    # partial pooled^T = (cbarT_slice^T @ wlv_rows)^T  [D, NB]
                    ppT = csb.tile([P, KT, NB], F32)
                    with tc.tile_pool(name="c_ps_pp", bufs=1, space="PSUM") as cps2:
                        for nch in range(D // 512):
                            pp_ps = cps2.tile([NB, 512], F32, tag="ppps", bufs=2)
                            for qq in range(SLT):
                                nc.tensor.matmul(pp_ps[:],
                                                 cT_loc[:, qq, :],
                                                 wlv_r[:, qq, nch * 512:(nch + 1) * 512],
                                                 start=(qq == 0), stop=(qq == SLT - 1),
                                                 skip_group_check=True)
                            pp_sb = csb.tile([NB, 512], F32, tag="ppsb", bufs=2)
                            nc.scalar.copy(out=pp_sb[:], in_=pp_ps[:])
                            for j in range(4):
                                pt_ps = cps2.tile([P, NB], F32, tag="ptps", bufs=2)
                                nc.tensor.matmul(pt_ps[:], pp_sb[:, j * P:(j + 1) * P],
                                                 ident_f[:NB, :NB], start=True, stop=True)
                                nc.scalar.copy(out=ppT[:, nch * 4 + j, :], in_=pt_ps[:])
                    nc.sync.dma_start(
                        pp_bounce[:].rearrange("(t p) b -> p t b", p=P), ppT[:])
                    nc.gpsimd.collective_compute(
                        "ReduceScatter", mybir.AluOpType.add, replica_groups=groups,
                        ins=[pp_bounce[:].opt()], outs=[ppr_bounce[:].opt()])
                    # out partial [NB, D] = pooledT_slice.T @ wout_rows + biasf
                    poT_f = csb.tile([P, SLT, NB], F32)
                    nc.sync.dma_start(
                        poT_f[:], ppr_bounce[:].rearrange("(t p) b -> p t b", p=P))

                    op_sb = csb.tile([NB, D], F32)
                    with tc.tile_pool(name="c_ps_op", bufs=1, space="PSUM") as cps3:
                        op_ps = cps3.tile([NB, D], F32)
                        for qq in range(SLT):
                            for nch in range(D // 512):
                                nc.tensor.matmul(op_ps[:, nch * 512:(nch + 1) * 512],
                                                 poT_f[:, qq, :],
                                                 wout_s[:, qq, nch * 512:(nch + 1) * 512],
                                                 start=(qq == 0), stop=False,
                                                 skip_group_check=True)
                        for nch in range(D // 512):
                            nc.tensor.matmul(op_ps[:, nch * 512:(nch + 1) * 512],
                                             ones1_f[:],
                                             biasf[:, nch * 512:(nch + 1) * 512],
                                             start=False, stop=(nch == D // 512 - 1),
                                             skip_group_check=True)
                        nc.scalar.copy(out=op_sb[:], in_=op_ps[:])
                    nc.sync.dma_start(op_bounce[:], op_sb[:])
                    nc.gpsimd.collective_compute(
                        "ReduceScatter", mybir.AluOpType.add, replica_groups=groups,
                        ins=[op_bounce[:].opt()], outs=[ob_bounce[:].opt()])
                    # broadcast own out row to 128 partitions
                    ob_sb = csb.tile([1, D], F32)
                    nc.sync.dma_start(ob_sb[:], ob_bounce[:])
                    obb = xq_pool.tile([P, D], F32)
                    with tc.tile_pool(name="c_ps_bc", bufs=1, space="PSUM") as cps4:
                        bc_ps = cps4.tile([P, D], F32)
                        for nch in range(D // 512):
                            nc.tensor.matmul(bc_ps[:, nch * 512:(nch + 1) * 512],
                                             onescol_f[:], ob_sb[:, nch * 512:(nch + 1) * 512],
                                             start=True, stop=True, skip_group_check=True)
                        nc.scalar.copy(out=obb[:], in_=bc_ps[:])

                # ================= PASS 2 (residual) =================
                with tc.tile_pool(name="res2", bufs=1) as r2:
                    for ti in range(NT):
                        if ti < NPF:
                            xin = pf[:, ti, :]
                        else:
                            xf2 = r2.tile([P, D], F32, tag="xf2", bufs=4)
                            nc.sync.dma_start(xf2[:], x_d[ti * P:(ti + 1) * P, :])
                            xin = xf2[:]
                        yt = r2.tile([P, D], F32, tag="yt", bufs=4)
                        nc.vector.tensor_tensor(yt[:], xin, obb[:], mybir.AluOpType.add)
                        nc.gpsimd.dma_start(y_d[ti * P:(ti + 1) * P, :], yt[:])
                pf_pool_ctx.__exit__(None, None, None)

    nc.compile()
    return nc


@functools.lru_cache(maxsize=2)
def _built(n_cores: int, S: int):
    return _build(n_cores, S)


def _host_prep(inputs, n_cores: int):
    """Weight folding on host. Returns (global_map, per_core_maps)."""
    NB = n_cores
    HPC = H // NB
    SL = D // NB
    SLT = SL // P
    f32 = np.float32
    bf16 = ml_dtypes.bfloat16

    x_all = np.ascontiguousarray(np.asarray(inputs["hidden_states"], dtype=f32))
    g = np.asarray(inputs["ln_g"], dtype=f32)
    b_ln = np.asarray(inputs["ln_b"], dtype=f32)
    lat = np.asarray(inputs["latents"], dtype=f32)
    w_lq = np.asarray(inputs["w_lq"], dtype=f32)
    b_lq = np.asarray(inputs["b_lq"], dtype=f32)
    w_k = np.asarray(inputs["w_k"], dtype=f32)
    w_v = np.asarray(inputs["w_v"], dtype=f32)
    b_v = np.asarray(inputs["b_v"], dtype=f32)
    w_lv = np.asarray(inputs["w_lv"], dtype=f32)
    b_lv = np.asarray(inputs["b_lv"], dtype=f32)
    w_out = np.asarray(inputs["w_out"], dtype=f32)
    b_out = np.asarray(inputs["b_out"], dtype=f32)

    q_full = lat @ w_lq + b_lq                      # [L, D]
    qhatT = np.empty((D, HL), f32)
    for h in range(H):
        qh = q_full[:, HD * h:HD * (h + 1)]          # [L, 128]
        qhatT[:, L * h:L * (h + 1)] = w_k[:, HD * h:HD * (h + 1)] @ qh.T
    qhatT *= g[:, None]
    c_vec = qhatT.sum(axis=0)                        # [HL]

    def tile_rows(a):  # [D, N] -> [P, KT, N] with d = t*128 + p
        return np.ascontiguousarray(a.reshape(KT, P, -1).transpose(1, 0, 2))

    qhatT_t = tile_rows(qhatT).astype(bf16)
    cneg = (-c_vec)[None, :].astype(bf16)

    selmat = np.zeros((P, 2, H), f32)
    for mh in range(2):
        for p in range(P):
            selmat[p, mh, (mh * P + p) // L] = 1.0 / L
    selmat = selmat.astype(f32)

    wv_g = w_v * g[:, None]
    bv_fold = b_v + b_ln @ w_v                       # [D]
    biasf_full = (b_lv @ w_out + b_out) / NB         # [D]

    global_map = {
        "qhatT": qhatT_t, "cneg": cneg, "selmat": selmat,
        "biasf": np.ascontiguousarray(biasf_full[None, :].astype(f32)),
    }
    per_core = []
    for c in range(NB):
        sl = slice(SL * c, SL * (c + 1))
        wv_s = tile_rows(wv_g[:, sl]).astype(f32)               # [P, KT, HPC*P]
        bv_row = bv_fold[None, sl].astype(f32)
        wlv_r = np.ascontiguousarray(
            w_lv[sl, :].reshape(SLT, P, D).transpose(1, 0, 2)).astype(f32)
        wout_s = np.ascontiguousarray(
            w_out[sl, :].reshape(SLT, P, D).transpose(1, 0, 2)).astype(f32)
        per_core.append({
            "x": np.ascontiguousarray(x_all[c]),
            "wv_s": wv_s, "bv_row": np.ascontiguousarray(bv_row),
            "wlv_r": wlv_r, "wout_s": wout_s,
        })
    return global_map, per_core


def kernel(**inputs) -> np.ndarray:
    NB = 8
    x_all = np.asarray(inputs["hidden_states"])
    B, S, D_ = x_all.shape
    assert D_ == D and B == NB
    nc = _built(NB, S)
    global_map, per_core = _host_prep(inputs, NB)
    in_maps = [{**global_map, **pc} for pc in per_core]
    res = run_bass_kernel_spmd(nc, in_maps, list(range(NB)))
    out = np.stack([res.results[i]["y"] for i in range(NB)], axis=0)
    return out.astype(np.float32)



# revision 7
# speedup vs baseline: 1.2737x; 1.2737x over previous
"""Trainium2 Bass kernel for nn_MultiLatentAttention (B=8, S=4096, D=2048, H=16, hd=128, L=16).

Strategy: data-parallel over batch (one batch element per core) with the giant
k/v projections algebraically eliminated, x kept resident in SBUF as bf16 so
the residual pass never re-reads HBM, and a 2-collective tail (AllToAll of the
tiny per-head context means + ReduceScatter of the rank-1 output row).

Math (per batch element):
  raw-x formulation: with alpha[t] = rsqrt(var[t]+eps), sig = 1/alpha,
    scoresT[t,hl]/sqrt(hd) = alpha[t] * (x[t,:].qhat_s[:,hl] - c_s[hl]*mu[t])
  where qhat_s = (Wk_head @ q) * g / sqrt(hd) folded host-side, c_s = sum_d.
  etilde = alpha * e = Exp(scale=alpha * psum + ln(alpha))   (one ACT op)
  u[hl,d] = sum_t etilde*x ; r = etilde^T@mu ; Z = etilde^T@sig  (=sum e)
  M = (u - r 1^T)/Z ; mbar = per-head mean over latents  [H, D]
  AllToAll routes heads {2c,2c+1} of every batch to core c; core applies its
  256-col slice of Wv*g and 256-row slice of W3 = Wlv@Wout (host-folded) for
  all 8 batches; ReduceScatter sums partials and lands row b on core b.
  y = x(bf16) + out  broadcast.  All biases folded host-side into one row.
"""

import sys
import functools
import numpy as np
import ml_dtypes

sys.path.insert(0, "/opt/trn_rl_repo")

import concourse.bass as bass
import concourse.mybir as mybir
import concourse.tile as tile
from concourse import bacc
from concourse.bass_utils import run_bass_kernel_spmd

BF = mybir.dt.bfloat16
F32 = mybir.dt.float32
AF = mybir.ActivationFunctionType

P = 128
D = 2048
KT = D // P          # 16 d-tiles
H = 16
HD = 128
L = 16
HL = H * L           # 256 score rows (h-major: hl = h*16 + l)
EPS = 1e-5
INV_SQRT_HD = 1.0 / float(np.sqrt(HD))


def _build(n_cores: int, S: int):
    NB = n_cores
    HPC = H // NB            # heads per core (2)
    SL = D // NB             # d_out slice width per core (256)
    NT = S // P              # token tiles (32)
    NQ = 4                   # u-sweep quarters
    TPQ = NT // NQ           # token tiles per quarter (8)
    NCH = D // 512           # 512-wide psum chunks (4)
    assert NT % NQ == 0 and H % NB == 0 and SL == HPC * P

    nc = bacc.Bacc(None, target_bir_lowering=False, num_devices=NB)
    groups = [list(range(NB))]

    with tile.TileContext(nc) as tc:
        with tc.tile_pool(name="dram", bufs=1, space="DRAM") as dram:
            def din(name, shape, dt):
                return dram.tile(shape, dt, kind="ExternalInput", name=name, uniquify=False)

            x_d = din("x", [S, D], F32)
            qhatT_d = din("qhatT", [P, KT, HL], BF)
            cneg_d = din("cneg", [1, HL], BF)
            selmat_d = din("selmat", [P, 2, H], F32)
            wvg_d = din("wvg_s", [P, KT, SL], BF)
            w3_d = din("w3_r", [P, HPC, D], BF)
            biasf_d = din("biasf", [1, D], BF)
            y_d = dram.tile([S, D], F32, kind="ExternalOutput", name="y", uniquify=False)

            # collective bounce buffers
            mb_bounce = dram.tile([H, D], BF, name="mb_bounce")
            m2_bounce = dram.tile([H, D], BF, name="m2_bounce")
            po_bounce = dram.tile([NB, D], F32, name="po_bounce")
            ob_bounce = dram.tile([1, D], F32, name="ob_bounce")

            with (
                tc.tile_pool(name="consts", bufs=1) as consts,
                tc.tile_pool(name="resident", bufs=1) as res,
            ):
                # ---- small constants ----
                qhatT = consts.tile([P, KT, HL], BF)
                nc.sync.dma_start(qhatT[:], qhatT_d[:])
                cneg = consts.tile([1, HL], BF)
                nc.sync.dma_start(cneg[:], cneg_d[:])
                selmat = consts.tile([P, 2, H], F32)
                nc.sync.dma_start(selmat[:], selmat_d[:])
                wvg_s = consts.tile([P, KT, SL], BF)
                nc.sync.dma_start(wvg_s[:], wvg_d[:])
                w3_r = consts.tile([P, HPC, D], BF)
                nc.sync.dma_start(w3_r[:], w3_d[:])
                biasf = consts.tile([1, D], BF)
                nc.sync.dma_start(biasf[:], biasf_d[:])

                ident_bf = consts.tile([P, P], BF)
                from concourse.masks import make_identity
                make_identity(nc, ident_bf)
                onesrow_bf = consts.tile([1, P], BF)
                nc.any.memset(onesrow_bf[:], 1.0)
                ones8_bf = consts.tile([1, NB], BF)
                nc.any.memset(ones8_bf[:], 1.0)
                eps_col = consts.tile([P, 1], F32)
                nc.any.memset(eps_col[:], EPS)

                # ---- persistent state ----
                xbf = res.tile([P, NT, D], BF)        # resident x (bf16)
                musig = res.tile([P, NT, 2], BF)      # [mu, sig] per token
                u_acc = res.tile([P, 2, D], F32)
                zr_acc = res.tile([P, 2, 2, NQ], F32)  # [mh, (r,Z), quarter]

                # ================= PASS 1 =================
                with (
                    tc.tile_pool(name="epool", bufs=1) as ep,
                    tc.tile_pool(name="p1sb", bufs=1) as sb,
                    tc.tile_pool(name="p1ps", bufs=1, space="PSUM") as ps,
                    tc.tile_pool(name="p1pu", bufs=1, space="PSUM") as psu,
                    tc.tile_pool(name="p1pzr", bufs=1, space="PSUM") as pszr,
                ):
                    etil = ep.tile([P, NT, HL], BF)   # etilde per token
                    for q in range(NQ):
                        for lt in range(TPQ):
                            ti = q * TPQ + lt
                            # stream x with f32->bf16 cast in the DMA
                            nc.gpsimd.dma_start(xbf[:, ti, :],
                                                x_d[ti * P:(ti + 1) * P, :])
                            # stats on bf16 x
                            bns = sb.tile([P, 4, 6], F32, tag="bns", bufs=3)
                            for a in range(4):
                                nc.vector.bn_stats(bns[:, a, :],
                                                   xbf[:, ti, a * 512:(a + 1) * 512])
                            mv = sb.tile([P, 2], F32, tag="mv", bufs=3)
                            nc.vector.bn_aggr(mv[:], bns[:])
                            sig = sb.tile([P, 1], F32, tag="sig", bufs=3)
                            nc.scalar.activation(sig[:], mv[:, 1:2], AF.Sqrt,
                                                 bias=eps_col[:])
                            alpha = sb.tile([P, 1], F32, tag="alpha", bufs=3)
                            nc.vector.reciprocal(alpha[:], sig[:])
                            lnal = sb.tile([P, 1], F32, tag="lnal", bufs=3)
                            nc.scalar.activation(lnal[:], alpha[:], AF.Ln)
                            nc.vector.tensor_copy(out=musig[:, ti, 0:1], in_=mv[:, 0:1])
                            nc.vector.tensor_copy(out=musig[:, ti, 1:2], in_=sig[:])
                            # transpose x tile -> [d, tok]
                            xbfT = sb.tile([P, KT, P], BF, tag="xbfT", bufs=3)
                            nc.sync.dma_start_transpose(xbfT[:], xbf[:, ti, :])
                            # mu as a row (PE transpose)
                            mur_ps = ps.tile([1, P], F32, tag="sc", bufs=3,
                                             name=f"mur{ti}")
                            nc.tensor.matmul(mur_ps[:], musig[:, ti, 0:1],
                                             ident_bf[:], start=True, stop=True)
                            murow = sb.tile([1, P], BF, tag="murow", bufs=3)
                            nc.scalar.copy(out=murow[:], in_=mur_ps[:])
                            # scoresT accumulation: rank-1 (-c*mu) then x.qhat
                            sc_ps = ps.tile([P, HL], F32, tag="sc", bufs=3,
                                            name=f"sc{ti}")
                            nc.tensor.matmul(sc_ps[:], murow[:], cneg[:],
                                             start=True, stop=False)
                            for kt in range(KT):
                                nc.tensor.matmul(sc_ps[:], xbfT[:, kt, :],
                                                 qhatT[:, kt, :],
                                                 start=False, stop=(kt == KT - 1))
                            # etilde = exp(alpha*s + ln(alpha))  (bf16)
                            nc.scalar.activation(etil[:, ti, :], sc_ps[:], AF.Exp,
                                                 scale=alpha[:], bias=lnal[:])

                        # ---- u / zr sweep for this quarter ----
                        for mh in range(2):
                            psum_u = psu.tile([P, D], F32, tag="u", bufs=1,
                                              name=f"u{q}_{mh}")
                            zr_ps = pszr.tile([P, 2], F32, tag="zr", bufs=1,
                                              name=f"zr{q}_{mh}")
                            for lt in range(TPQ):
                                ti = q * TPQ + lt
                                lhs = etil[:, ti, mh * P:(mh + 1) * P]
                                for nch in range(NCH):
                                    nc.tensor.matmul(
                                        psum_u[:, nch * 512:(nch + 1) * 512],
                                        lhs, xbf[:, ti, nch * 512:(nch + 1) * 512],
                                        start=(lt == 0), stop=(lt == TPQ - 1),
                                        skip_group_check=True)
                                nc.tensor.matmul(zr_ps[:], lhs, musig[:, ti, :],
                                                 start=(lt == 0), stop=(lt == TPQ - 1),
                                                 skip_group_check=True)
                            if q == 0:
                                nc.scalar.copy(out=u_acc[:, mh, :], in_=psum_u[:])
                            else:
                                nc.vector.tensor_tensor(u_acc[:, mh, :],
                                                        u_acc[:, mh, :], psum_u[:],
                                                        mybir.AluOpType.add)
                            nc.scalar.copy(out=zr_acc[:, mh, :, q], in_=zr_ps[:])

                # ================= STAGE C =================
                with tc.tile_pool(name="c_sb", bufs=1) as csb:
                    # r, Z totals; M' = (u - r)/Z in place
                    zrt = csb.tile([P, 2, 2], F32)
                    nc.vector.tensor_reduce(zrt[:], zr_acc[:], mybir.AxisListType.X,
                                            mybir.AluOpType.add)
                    rzi = csb.tile([P, 2, 1], F32)
                    nc.vector.reciprocal(rzi[:], zrt[:, :, 1:2])
                    for mh in range(2):
                        nc.vector.tensor_scalar(u_acc[:, mh, :], u_acc[:, mh, :],
                                                zrt[:, mh, 0:1], rzi[:, mh, :],
                                                mybir.AluOpType.subtract,
                                                mybir.AluOpType.mult)
                    # mbar = per-head mean [H, D] (bf16)
                    mb_bf = csb.tile([H, D], BF)
                    with tc.tile_pool(name="c_ps_mb", bufs=1, space="PSUM") as cps0:
                        mb_ps = cps0.tile([H, D], F32)
                        for mh in range(2):
                            for nch in range(NCH):
                                nc.tensor.matmul(mb_ps[:, nch * 512:(nch + 1) * 512],
                                                 selmat[:, mh, :],
                                                 u_acc[:, mh, nch * 512:(nch + 1) * 512],
                                                 start=(mh == 0), stop=(mh == 1),
                                                 skip_group_check=True)
                        nc.scalar.copy(out=mb_bf[:], in_=mb_ps[:])
                    nc.sync.dma_start(mb_bounce[:], mb_bf[:])
                    nc.gpsimd.collective_compute(
                        "AllToAll", mybir.AluOpType.bypass, replica_groups=groups,
                        ins=[mb_bounce[:].opt()], outs=[m2_bounce[:].opt()])
                    # m2 rows = (batch b, local head h2); transpose -> [d, (b,h2)]
                    m2T = csb.tile([P, KT, H], BF)
                    nc.sync.dma_start_transpose(m2T[:], m2_bounce[:])
                    # cb[h2] = mb_h @ Wvg_slice block  -> transpose -> [j, b]
                    cbT = csb.tile([P, HPC, NB], BF)
                    with tc.tile_pool(name="c_ps_cb", bufs=1, space="PSUM") as cps1:
                        for h2 in range(HPC):
                            cb_ps = cps1.tile([NB, P], F32, tag="cbps", bufs=2)
                            for kt in range(KT):
                                lhs = m2T[:, kt, :].rearrange("p (b h) -> p h b", h=HPC)
                                nc.tensor.matmul(cb_ps[:], lhs[:, h2, :],
                                                 wvg_s[:, kt, h2 * P:(h2 + 1) * P],
                                                 start=(kt == 0), stop=(kt == KT - 1))
                            cb_sb = csb.tile([NB, P], BF, tag="cbsb", bufs=2)
                            nc.scalar.copy(out=cb_sb[:], in_=cb_ps[:])
                            ct_ps = cps1.tile([P, NB], F32, tag="ctps", bufs=2)
                            nc.tensor.matmul(ct_ps[:], cb_sb[:], ident_bf[:NB, :NB],
                                             start=True, stop=True)
                            nc.scalar.copy(out=cbT[:, h2, :], in_=ct_ps[:])
                    # partial out rows for all batches: po = cb @ W3_rows + biasf/NB
                    with tc.tile_pool(name="c_ps_po", bufs=1, space="PSUM") as cps2:
                        po_ps = cps2.tile([NB, D], F32)
                        for h2 in range(HPC):
                            for nch in range(NCH):
                                nc.tensor.matmul(po_ps[:, nch * 512:(nch + 1) * 512],
                                                 cbT[:, h2, :],
                                                 w3_r[:, h2, nch * 512:(nch + 1) * 512],
                                                 start=(h2 == 0), stop=False,
                                                 skip_group_check=True)
                        for nch in range(NCH):
                            nc.tensor.matmul(po_ps[:, nch * 512:(nch + 1) * 512],
                                             ones8_bf[:],
                                             biasf[:, nch * 512:(nch + 1) * 512],
                                             start=False, stop=(nch == NCH - 1),
                                             skip_group_check=True)
                        for half in range(2):
                            po_sb = csb.tile([NB, D // 2], F32, tag="posb", bufs=1)
                            nc.scalar.copy(out=po_sb[:],
                                           in_=po_ps[:, half * 1024:(half + 1) * 1024])
                            nc.sync.dma_start(
                                po_bounce[:, half * 1024:(half + 1) * 1024], po_sb[:])
                    nc.gpsimd.collective_compute(
                        "ReduceScatter", mybir.AluOpType.add, replica_groups=groups,
                        ins=[po_bounce[:].opt()], outs=[ob_bounce[:].opt()])
                    # broadcast own out row to 128 partitions (bf16)
                    ob_bf = csb.tile([1, D], BF)
                    nc.gpsimd.dma_start(ob_bf[:], ob_bounce[:])  # f32->bf16 cast
                    obb = res.tile([P, D], BF)
                    with tc.tile_pool(name="c_ps_bc", bufs=1, space="PSUM") as cps3:
                        bc_ps = cps3.tile([P, D], F32)
                        for nch in range(NCH):
                            nc.tensor.matmul(bc_ps[:, nch * 512:(nch + 1) * 512],
                                             onesrow_bf[:],
                                             ob_bf[:, nch * 512:(nch + 1) * 512],
                                             start=True, stop=True,
                                             skip_group_check=True)
                        nc.scalar.copy(out=obb[:], in_=bc_ps[:])

                # ================= PASS 2 (residual, no x re-read) =========
                with tc.tile_pool(name="res2", bufs=1) as r2:
                    for ti in range(NT):
                        yt = r2.tile([P, D], F32, tag="yt", bufs=3)
                        nc.vector.tensor_tensor(yt[:], xbf[:, ti, :], obb[:],
                                                mybir.AluOpType.add)
                        nc.sync.dma_start(y_d[ti * P:(ti + 1) * P, :], yt[:])

    nc.compile()
    return nc


@functools.lru_cache(maxsize=2)
def _built(n_cores: int, S: int):
    return _build(n_cores, S)


def _host_prep(inputs, n_cores: int):
    """Weight folding on host. Returns (global_map, per_core_maps)."""
    NB = n_cores
    HPC = H // NB
    SL = D // NB
    f32 = np.float32
    bf16 = ml_dtypes.bfloat16

    x_all = np.ascontiguousarray(np.asarray(inputs["hidden_states"], dtype=f32))
    g = np.asarray(inputs["ln_g"], dtype=f32)
    b_ln = np.asarray(inputs["ln_b"], dtype=f32)
    lat = np.asarray(inputs["latents"], dtype=f32)
    w_lq = np.asarray(inputs["w_lq"], dtype=f32)
    b_lq = np.asarray(inputs["b_lq"], dtype=f32)
    w_k = np.asarray(inputs["w_k"], dtype=f32)
    w_v = np.asarray(inputs["w_v"], dtype=f32)
    b_v = np.asarray(inputs["b_v"], dtype=f32)
    w_lv = np.asarray(inputs["w_lv"], dtype=f32)
    b_lv = np.asarray(inputs["b_lv"], dtype=f32)
    w_out = np.asarray(inputs["w_out"], dtype=f32)
    b_out = np.asarray(inputs["b_out"], dtype=f32)

    q_full = lat @ w_lq + b_lq                      # [L, D]
    qhatT = np.empty((D, HL), f32)
    for h in range(H):
        qh = q_full[:, HD * h:HD * (h + 1)]          # [L, 128]
        qhatT[:, L * h:L * (h + 1)] = w_k[:, HD * h:HD * (h + 1)] @ qh.T
    qhatT *= g[:, None] * INV_SQRT_HD               # fold 1/sqrt(hd)
    c_vec = qhatT.sum(axis=0)                        # [HL] (already scaled)

    def tile_rows(a):  # [D, N] -> [P, KT, N] with d = t*128 + p
        return np.ascontiguousarray(a.reshape(-1, P, a.shape[-1]).transpose(1, 0, 2))

    qhatT_t = tile_rows(qhatT).astype(bf16)
    cneg = (-c_vec)[None, :].astype(bf16)

    selmat = np.zeros((P, 2, H), f32)
    for mh in range(2):
        for p in range(P):
            selmat[p, mh, (mh * P + p) // L] = 1.0 / L

    wvg = w_v * g[:, None]
    w3 = w_lv @ w_out                                # folded Wlv@Wout [D, D]
    bv_fold = b_v + b_ln @ w_v
    biasf_full = ((bv_fold @ w_lv + b_lv) @ w_out + b_out) / NB

    global_map = {
        "qhatT": qhatT_t, "cneg": cneg, "selmat": selmat,
        "biasf": np.ascontiguousarray(biasf_full[None, :].astype(bf16)),
    }
    per_core = []
    for c in range(NB):
        sl = slice(SL * c, SL * (c + 1))
        wvg_s = tile_rows(wvg[:, sl]).astype(bf16)               # [P, KT, SL]
        w3_rows = np.ascontiguousarray(
            w3[sl, :].reshape(HPC, P, D).transpose(1, 0, 2)).astype(bf16)
        per_core.append({
            "x": np.ascontiguousarray(x_all[c]),
            "wvg_s": wvg_s, "w3_r": w3_rows,
        })
    return global_map, per_core


def kernel(**inputs) -> np.ndarray:
    NB = 8
    x_all = np.asarray(inputs["hidden_states"])
    B, S, D_ = x_all.shape
    assert D_ == D and B == NB
    nc = _built(NB, S)
    global_map, per_core = _host_prep(inputs, NB)
    in_maps = [{**global_map, **pc} for pc in per_core]
    res = run_bass_kernel_spmd(nc, in_maps, list(range(NB)))
    out = np.stack([res.results[i]["y"] for i in range(NB)], axis=0)
    return out.astype(np.float32)


# revision 13
# speedup vs baseline: 1.2900x; 1.0128x over previous
"""Trainium2 Bass kernel for nn_MultiLatentAttention (B=8, S=4096, D=2048, H=16, hd=128, L=16).

Strategy: data-parallel over batch (one batch element per core) with the giant
k/v projections algebraically eliminated, x kept resident in SBUF as bf16 so
the residual pass never re-reads HBM, and a 2-collective tail (AllToAll of the
tiny per-head context means + ReduceScatter of the rank-1 output row).

Math (per batch element):
  raw-x formulation: with alpha[t] = rsqrt(var[t]+eps), sig = 1/alpha,
    scoresT[t,hl]/sqrt(hd) = alpha[t] * (x[t,:].qhat_s[:,hl] - c_s[hl]*mu[t])
  where qhat_s = (Wk_head @ q) * g / sqrt(hd) folded host-side, c_s = sum_d.
  etilde = alpha * e = Exp(scale=alpha * psum + ln(alpha))   (one ACT op)
  u[hl,d] = sum_t etilde*x ; r = etilde^T@mu ; Z = etilde^T@sig  (=sum e)
  M = (u - r 1^T)/Z ; mbar = per-head mean over latents  [H, D]
  AllToAll routes heads {2c,2c+1} of every batch to core c; core applies its
  256-col slice of Wv*g and 256-row slice of W3 = Wlv@Wout (host-folded) for
  all 8 batches; ReduceScatter sums partials and lands row b on core b.
  y = x(bf16) + out  broadcast.  All biases folded host-side into one row.
"""

import sys
import functools
import numpy as np
import ml_dtypes

sys.path.insert(0, "/opt/trn_rl_repo")

import concourse.bass as bass
import concourse.mybir as mybir
import concourse.tile as tile
from concourse import bacc
from concourse.bass_utils import run_bass_kernel_spmd

BF = mybir.dt.bfloat16
F32 = mybir.dt.float32
AF = mybir.ActivationFunctionType

P = 128
D = 2048
KT = D // P          # 16 d-tiles
H = 16
HD = 128
L = 16
HL = H * L           # 256 score rows (h-major: hl = h*16 + l)
EPS = 1e-5
INV_SQRT_HD = 1.0 / float(np.sqrt(HD))


def _build(n_cores: int, S: int):
    NB = n_cores
    HPC = H // NB            # heads per core (2)
    SL = D // NB             # d_out slice width per core (256)
    NT = S // P              # token tiles (32)
    NQ = 4                   # u-sweep quarters
    TPQ = NT // NQ           # token tiles per quarter (8)
    NCH = D // 512           # 512-wide psum chunks (4)
    assert NT % NQ == 0 and H % NB == 0 and SL == HPC * P

    nc = bacc.Bacc(None, target_bir_lowering=False, num_devices=NB)
    groups = [list(range(NB))]

    with tile.TileContext(nc) as tc:
        with tc.tile_pool(name="dram", bufs=1, space="DRAM") as dram:
            def din(name, shape, dt):
                return dram.tile(shape, dt, kind="ExternalInput", name=name, uniquify=False)

            x_d = din("x", [S, D], F32)
            qhatT_d = din("qhatT", [P, KT, HL], BF)
            cneg_d = din("cneg", [1, HL], BF)
            selmat_d = din("selmat", [P, 2, H], F32)
            wvg_d = din("wvg_s", [P, KT, SL], BF)
            w3_d = din("w3_r", [P, HPC, D], BF)
            biasf_d = din("biasf", [1, D], BF)
            y_d = dram.tile([S, D], F32, kind="ExternalOutput", name="y", uniquify=False)

            # collective bounce buffers
            mb_bounce = dram.tile([H, D], BF, name="mb_bounce")
            m2_bounce = dram.tile([H, D], BF, name="m2_bounce")
            po_bounce = dram.tile([NB, D], F32, name="po_bounce")
            ob_bounce = dram.tile([1, D], F32, name="ob_bounce")

            with (
                tc.tile_pool(name="consts", bufs=1) as consts,
                tc.tile_pool(name="resident", bufs=1) as res,
            ):
                # ---- small constants ----
                qhatT = consts.tile([P, KT, HL], BF)
                nc.sync.dma_start(qhatT[:], qhatT_d[:])
                cneg = consts.tile([1, HL], BF)
                nc.sync.dma_start(cneg[:], cneg_d[:])
                selmat = consts.tile([P, 2, H], F32)
                nc.sync.dma_start(selmat[:], selmat_d[:])
                wvg_s = consts.tile([P, KT, SL], BF)
                nc.sync.dma_start(wvg_s[:], wvg_d[:])
                w3_r = consts.tile([P, HPC, D], BF)
                nc.sync.dma_start(w3_r[:], w3_d[:])
                biasf = consts.tile([1, D], BF)
                nc.sync.dma_start(biasf[:], biasf_d[:])

                ident_bf = consts.tile([P, P], BF)
                from concourse.masks import make_identity
                make_identity(nc, ident_bf)
                onesrow_bf = consts.tile([1, P], BF)
                nc.any.memset(onesrow_bf[:], 1.0)
                ones8_bf = consts.tile([1, NB], BF)
                nc.any.memset(ones8_bf[:], 1.0)
                eps_col = consts.tile([P, 1], F32)
                nc.any.memset(eps_col[:], EPS)

                # ---- persistent state ----
                xbf = res.tile([P, NT, D], BF)        # resident x (bf16)
                musig = res.tile([P, NT, 2], BF)      # [mu, sig] per token
                u_acc = res.tile([P, 2, D], F32)
                zr_acc = res.tile([P, 2, 2, NQ], F32)  # [mh, (r,Z), quarter]

                # ================= PASS 1 =================
                with (
                    tc.tile_pool(name="epool", bufs=1) as ep,
                    tc.tile_pool(name="p1sb", bufs=1) as sb,
                    tc.tile_pool(name="p1ps", bufs=1, space="PSUM") as ps,
                    tc.tile_pool(name="p1pu", bufs=1, space="PSUM") as psu,
                    tc.tile_pool(name="p1pzr", bufs=1, space="PSUM") as pszr,
                ):
                    etil = ep.tile([P, NT, HL], BF)   # etilde per token
                    # stream all of x up front (resident; cast f32->bf16 in DMA)
                    for ti in range(NT):
                        nc.gpsimd.dma_start(xbf[:, ti, :],
                                            x_d[ti * P:(ti + 1) * P, :])
                    for q in range(NQ):
                        for lt in range(TPQ):
                            ti = q * TPQ + lt
                            # stats on bf16 x
                            bns = sb.tile([P, 4, 6], F32, tag="bns", bufs=3)
                            for a in range(4):
                                nc.vector.bn_stats(bns[:, a, :],
                                                   xbf[:, ti, a * 512:(a + 1) * 512])
                            mv = sb.tile([P, 2], F32, tag="mv", bufs=3)
                            nc.vector.bn_aggr(mv[:], bns[:])
                            sig = sb.tile([P, 1], F32, tag="sig", bufs=3)
                            nc.scalar.activation(sig[:], mv[:, 1:2], AF.Sqrt,
                                                 bias=eps_col[:])
                            alpha = sb.tile([P, 1], F32, tag="alpha", bufs=3)
                            nc.vector.reciprocal(alpha[:], sig[:])
                            nc.vector.tensor_copy(out=musig[:, ti, 0:1], in_=mv[:, 0:1])
                            nc.vector.tensor_copy(out=musig[:, ti, 1:2], in_=sig[:])
                            # transpose x tile -> [d, tok]
                            xbfT = sb.tile([P, KT, P], BF, tag="xbfT", bufs=3)
                            nc.sync.dma_start_transpose(xbfT[:], xbf[:, ti, :])
                            # mu as a row (PE transpose)
                            mur_ps = ps.tile([1, P], F32, tag="sc", bufs=3,
                                             name=f"mur{ti}")
                            nc.tensor.matmul(mur_ps[:], musig[:, ti, 0:1],
                                             ident_bf[:], start=True, stop=True)
                            murow = sb.tile([1, P], BF, tag="murow", bufs=3)
                            nc.vector.tensor_copy(out=murow[:], in_=mur_ps[:])
                            # scoresT accumulation: rank-1 (-c*mu) then x.qhat
                            sc_ps = ps.tile([P, HL], F32, tag="sc", bufs=3,
                                            name=f"sc{ti}")
                            nc.tensor.matmul(sc_ps[:], murow[:], cneg[:],
                                             start=True, stop=False)
                            for kt in range(KT):
                                nc.tensor.matmul(sc_ps[:], xbfT[:, kt, :],
                                                 qhatT[:, kt, :],
                                                 start=False, stop=(kt == KT - 1))
                            # etilde = alpha * exp(alpha*s)  (bf16)
                            eraw = sb.tile([P, HL], BF, tag="eraw", bufs=2)
                            nc.scalar.activation(eraw[:], sc_ps[:], AF.Exp,
                                                 scale=alpha[:])
                            nc.vector.tensor_scalar_mul(etil[:, ti, :], eraw[:],
                                                        alpha[:])

                        # ---- u / zr sweep for this quarter ----
                        for mh in range(2):
                            psum_u = psu.tile([P, D], F32, tag="u", bufs=1,
                                              name=f"u{q}_{mh}")
                            zr_ps = pszr.tile([P, 2], F32, tag="zr", bufs=1,
                                              name=f"zr{q}_{mh}")
                            for lt in range(TPQ):
                                ti = q * TPQ + lt
                                lhs = etil[:, ti, mh * P:(mh + 1) * P]
                                for nch in range(NCH):
                                    nc.tensor.matmul(
                                        psum_u[:, nch * 512:(nch + 1) * 512],
                                        lhs, xbf[:, ti, nch * 512:(nch + 1) * 512],
                                        start=(lt == 0), stop=(lt == TPQ - 1),
                                        skip_group_check=True)
                                nc.tensor.matmul(zr_ps[:], lhs, musig[:, ti, :],
                                                 start=(lt == 0), stop=(lt == TPQ - 1),
                                                 skip_group_check=True)
                            if q == 0:
                                nc.vector.tensor_copy(out=u_acc[:, mh, :],
                                                      in_=psum_u[:])
                            else:
                                nc.vector.tensor_tensor(u_acc[:, mh, :],
                                                        u_acc[:, mh, :], psum_u[:],
                                                        mybir.AluOpType.add)
                            nc.vector.tensor_copy(out=zr_acc[:, mh, :, q],
                                                  in_=zr_ps[:])

                # ================= STAGE C =================
                with tc.tile_pool(name="c_sb", bufs=1) as csb:
                    # r, Z totals; M' = (u - r)/Z in place
                    zrt = csb.tile([P, 2, 2], F32)
                    nc.vector.tensor_reduce(zrt[:], zr_acc[:], mybir.AxisListType.X,
                                            mybir.AluOpType.add)
                    rzi = csb.tile([P, 2, 1], F32)
                    nc.vector.reciprocal(rzi[:], zrt[:, :, 1:2])
                    for mh in range(2):
                        nc.vector.tensor_scalar(u_acc[:, mh, :], u_acc[:, mh, :],
                                                zrt[:, mh, 0:1], rzi[:, mh, :],
                                                mybir.AluOpType.subtract,
                                                mybir.AluOpType.mult)
                    # mbar = per-head mean [H, D] (bf16)
                    mb_bf = csb.tile([H, D], BF)
                    with tc.tile_pool(name="c_ps_mb", bufs=1, space="PSUM") as cps0:
                        mb_ps = cps0.tile([H, D], F32)
                        for mh in range(2):
                            for nch in range(NCH):
                                nc.tensor.matmul(mb_ps[:, nch * 512:(nch + 1) * 512],
                                                 selmat[:, mh, :],
                                                 u_acc[:, mh, nch * 512:(nch + 1) * 512],
                                                 start=(mh == 0), stop=(mh == 1),
                                                 skip_group_check=True)
                        nc.scalar.copy(out=mb_bf[:], in_=mb_ps[:])
                    nc.sync.dma_start(mb_bounce[:], mb_bf[:])
                    nc.gpsimd.collective_compute(
                        "AllToAll", mybir.AluOpType.bypass, replica_groups=groups,
                        ins=[mb_bounce[:].opt()], outs=[m2_bounce[:].opt()])
                    # m2 rows = (batch b, local head h2); transpose -> [d, (b,h2)]
                    m2T = csb.tile([P, KT, H], BF)
                    nc.sync.dma_start_transpose(m2T[:], m2_bounce[:])
                    # cb[h2] = mb_h @ Wvg_slice block  -> transpose -> [j, b]
                    cbT = csb.tile([P, HPC, NB], BF)
                    with tc.tile_pool(name="c_ps_cb", bufs=1, space="PSUM") as cps1:
                        for h2 in range(HPC):
                            cb_ps = cps1.tile([NB, P], F32, tag="cbps", bufs=2)
                            for kt in range(KT):
                                lhs = m2T[:, kt, :].rearrange("p (b h) -> p h b", h=HPC)
                                nc.tensor.matmul(cb_ps[:], lhs[:, h2, :],
                                                 wvg_s[:, kt, h2 * P:(h2 + 1) * P],
                                                 start=(kt == 0), stop=(kt == KT - 1))
                            cb_sb = csb.tile([NB, P], BF, tag="cbsb", bufs=2)
                            nc.scalar.copy(out=cb_sb[:], in_=cb_ps[:])
                            ct_ps = cps1.tile([P, NB], F32, tag="ctps", bufs=2)
                            nc.tensor.matmul(ct_ps[:], cb_sb[:], ident_bf[:NB, :NB],
                                             start=True, stop=True)
                            nc.scalar.copy(out=cbT[:, h2, :], in_=ct_ps[:])
                    # partial out rows for all batches: po = cb @ W3_rows + biasf/NB
                    with tc.tile_pool(name="c_ps_po", bufs=1, space="PSUM") as cps2:
                        po_ps = cps2.tile([NB, D], F32)
                        for h2 in range(HPC):
                            for nch in range(NCH):
                                nc.tensor.matmul(po_ps[:, nch * 512:(nch + 1) * 512],
                                                 cbT[:, h2, :],
                                                 w3_r[:, h2, nch * 512:(nch + 1) * 512],
                                                 start=(h2 == 0), stop=False,
                                                 skip_group_check=True)
                        for nch in range(NCH):
                            nc.tensor.matmul(po_ps[:, nch * 512:(nch + 1) * 512],
                                             ones8_bf[:],
                                             biasf[:, nch * 512:(nch + 1) * 512],
                                             start=False, stop=(nch == NCH - 1),
                                             skip_group_check=True)
                        for half in range(2):
                            po_sb = csb.tile([NB, D // 2], F32, tag="posb", bufs=1)
                            nc.scalar.copy(out=po_sb[:],
                                           in_=po_ps[:, half * 1024:(half + 1) * 1024])
                            nc.sync.dma_start(
                                po_bounce[:, half * 1024:(half + 1) * 1024], po_sb[:])
                    nc.gpsimd.collective_compute(
                        "ReduceScatter", mybir.AluOpType.add, replica_groups=groups,
                        ins=[po_bounce[:].opt()], outs=[ob_bounce[:].opt()])
                    # broadcast own out row to 128 partitions (bf16)
                    ob_bf = csb.tile([1, D], BF)
                    nc.gpsimd.dma_start(ob_bf[:], ob_bounce[:])  # f32->bf16 cast
                    obb = res.tile([P, D], BF)
                    with tc.tile_pool(name="c_ps_bc", bufs=1, space="PSUM") as cps3:
                        bc_ps = cps3.tile([P, D], F32)
                        for nch in range(NCH):
                            nc.tensor.matmul(bc_ps[:, nch * 512:(nch + 1) * 512],
                                             onesrow_bf[:],
                                             ob_bf[:, nch * 512:(nch + 1) * 512],
                                             start=True, stop=True,
                                             skip_group_check=True)
                        nc.scalar.copy(out=obb[:], in_=bc_ps[:])

                # ================= PASS 2 (residual, no x re-read) =========
                with tc.tile_pool(name="res2", bufs=1) as r2:
                    for ti in range(NT):
                        yt = r2.tile([P, D], F32, tag="yt", bufs=3)
                        nc.vector.tensor_tensor(yt[:], xbf[:, ti, :], obb[:],
                                                mybir.AluOpType.add)
                        nc.sync.dma_start(y_d[ti * P:(ti + 1) * P, :], yt[:])

    nc.compile()
    return nc


@functools.lru_cache(maxsize=2)
def _built(n_cores: int, S: int):
    return _build(n_cores, S)


def _host_prep(inputs, n_cores: int):
    """Weight folding on host. Returns (global_map, per_core_maps)."""
    NB = n_cores
    HPC = H // NB
    SL = D // NB
    f32 = np.float32
    bf16 = ml_dtypes.bfloat16

    x_all = np.ascontiguousarray(np.asarray(inputs["hidden_states"], dtype=f32))
    g = np.asarray(inputs["ln_g"], dtype=f32)
    b_ln = np.asarray(inputs["ln_b"], dtype=f32)
    lat = np.asarray(inputs["latents"], dtype=f32)
    w_lq = np.asarray(inputs["w_lq"], dtype=f32)
    b_lq = np.asarray(inputs["b_lq"], dtype=f32)
    w_k = np.asarray(inputs["w_k"], dtype=f32)
    w_v = np.asarray(inputs["w_v"], dtype=f32)
    b_v = np.asarray(inputs["b_v"], dtype=f32)
    w_lv = np.asarray(inputs["w_lv"], dtype=f32)
    b_lv = np.asarray(inputs["b_lv"], dtype=f32)
    w_out = np.asarray(inputs["w_out"], dtype=f32)
    b_out = np.asarray(inputs["b_out"], dtype=f32)

    q_full = lat @ w_lq + b_lq                      # [L, D]
    qhatT = np.empty((D, HL), f32)
    for h in range(H):
        qh = q_full[:, HD * h:HD * (h + 1)]          # [L, 128]
        qhatT[:, L * h:L * (h + 1)] = w_k[:, HD * h:HD * (h + 1)] @ qh.T
    qhatT *= g[:, None] * INV_SQRT_HD               # fold 1/sqrt(hd)
    c_vec = qhatT.sum(axis=0)                        # [HL] (already scaled)

    def tile_rows(a):  # [D, N] -> [P, KT, N] with d = t*128 + p
        return np.ascontiguousarray(a.reshape(-1, P, a.shape[-1]).transpose(1, 0, 2))

    qhatT_t = tile_rows(qhatT).astype(bf16)
    cneg = (-c_vec)[None, :].astype(bf16)

    selmat = np.zeros((P, 2, H), f32)
    for mh in range(2):
        for p in range(P):
            selmat[p, mh, (mh * P + p) // L] = 1.0 / L

    wvg = w_v * g[:, None]
    w3 = w_lv @ w_out                                # folded Wlv@Wout [D, D]
    bv_fold = b_v + b_ln @ w_v
    biasf_full = ((bv_fold @ w_lv + b_lv) @ w_out + b_out) / NB

    global_map = {
        "qhatT": qhatT_t, "cneg": cneg, "selmat": selmat,
        "biasf": np.ascontiguousarray(biasf_full[None, :].astype(bf16)),
    }
    per_core = []
    for c in range(NB):
        sl = slice(SL * c, SL * (c + 1))
        wvg_s = tile_rows(wvg[:, sl]).astype(bf16)               # [P, KT, SL]
        w3_rows = np.ascontiguousarray(
            w3[sl, :].reshape(HPC, P, D).transpose(1, 0, 2)).astype(bf16)
        per_core.append({
            "x": np.ascontiguousarray(x_all[c]),
            "wvg_s": wvg_s, "w3_r": w3_rows,
        })
    return global_map, per_core


def kernel(**inputs) -> np.ndarray:
    NB = 8
    x_all = np.asarray(inputs["hidden_states"])
    B, S, D_ = x_all.shape
    assert D_ == D and B == NB
    nc = _built(NB, S)
    global_map, per_core = _host_prep(inputs, NB)
    in_maps = [{**global_map, **pc} for pc in per_core]
    res = run_bass_kernel_spmd(nc, in_maps, list(range(NB)))
    out = np.stack([res.results[i]["y"] for i in range(NB)], axis=0)
    return out.astype(np.float32)


# revision 17
# speedup vs baseline: 1.5081x; 1.1690x over previous
"""Trainium2 Bass kernel for nn_MultiLatentAttention (B=8, S=4096, D=2048, H=16, hd=128, L=16).

Strategy: data-parallel over batch (one batch element per core) with the giant
k/v projections algebraically eliminated, x kept resident in SBUF as bf16 so
the residual pass never re-reads HBM, and a 2-collective tail (AllToAll of the
tiny per-head context means + ReduceScatter of the rank-1 output row).

Math (per batch element):
  raw-x formulation: with alpha[t] = rsqrt(var[t]+eps), sig = 1/alpha,
    scoresT[t,hl]/sqrt(hd) = alpha[t] * (x[t,:].qhat_s[:,hl] - c_s[hl]*mu[t])
  where qhat_s = (Wk_head @ q) * g / sqrt(hd) folded host-side, c_s = sum_d.
  etilde = alpha * e = Exp(scale=alpha * psum + ln(alpha))   (one ACT op)
  u[hl,d] = sum_t etilde*x ; r = etilde^T@mu ; Z = etilde^T@sig  (=sum e)
  M = (u - r 1^T)/Z ; mbar = per-head mean over latents  [H, D]
  AllToAll routes heads {2c,2c+1} of every batch to core c; core applies its
  256-col slice of Wv*g and 256-row slice of W3 = Wlv@Wout (host-folded) for
  all 8 batches; ReduceScatter sums partials and lands row b on core b.
  y = x(bf16) + out  broadcast.  All biases folded host-side into one row.
"""

import sys
import functools
import numpy as np
import ml_dtypes

sys.path.insert(0, "/opt/trn_rl_repo")

import concourse.bass as bass
import concourse.mybir as mybir
import concourse.tile as tile
from concourse import bacc
from concourse.bass_utils import run_bass_kernel_spmd

BF = mybir.dt.bfloat16
F32 = mybir.dt.float32
AF = mybir.ActivationFunctionType

P = 128
D = 2048
KT = D // P          # 16 d-tiles
H = 16
HD = 128
L = 16
HL = H * L           # 256 score rows (h-major: hl = h*16 + l)
EPS = 1e-5
INV_SQRT_HD = 1.0 / float(np.sqrt(HD))


def _build(n_cores: int, S: int):
    NB = n_cores
    HPC = H // NB            # heads per core (2)
    SL = D // NB             # d_out slice width per core (256)
    NT = S // P              # token tiles (32)
    NQ = 4                   # u-sweep quarters
    TPQ = NT // NQ           # token tiles per quarter (8)
    NCH = D // 512           # 512-wide psum chunks (4)
    assert NT % NQ == 0 and H % NB == 0 and SL == HPC * P

    nc = bacc.Bacc(None, target_bir_lowering=False, num_devices=NB)
    groups = [list(range(NB))]

    with tile.TileContext(nc) as tc:
        with tc.tile_pool(name="dram", bufs=1, space="DRAM") as dram:
            def din(name, shape, dt):
                return dram.tile(shape, dt, kind="ExternalInput", name=name, uniquify=False)

            x_d = din("x", [S, D], BF)
            qhatT_d = din("qhatT", [P, KT, HL], BF)
            cneg_d = din("cneg", [1, HL], BF)
            selmat_d = din("selmat", [P, 2, H], F32)
            wvg_d = din("wvg_s", [P, KT, SL], BF)
            w3_d = din("w3_r", [P, HPC, D], BF)
            biasf_d = din("biasf", [1, D], BF)
            y_d = dram.tile([S, D], F32, kind="ExternalOutput", name="y", uniquify=False)

            # collective bounce buffers
            mb_bounce = dram.tile([H, D], BF, name="mb_bounce")
            m2_bounce = dram.tile([H, D], BF, name="m2_bounce")
            po_bounce = dram.tile([NB, D], F32, name="po_bounce")
            ob_bounce = dram.tile([1, D], F32, name="ob_bounce")

            with (
                tc.tile_pool(name="consts", bufs=1) as consts,
                tc.tile_pool(name="resident", bufs=1) as res,
            ):
                # ---- small constants ----
                qhatT = consts.tile([P, KT, HL], BF)
                nc.sync.dma_start(qhatT[:], qhatT_d[:])
                cneg = consts.tile([1, HL], BF)
                nc.sync.dma_start(cneg[:], cneg_d[:])
                selmat = consts.tile([P, 2, H], F32)
                nc.sync.dma_start(selmat[:], selmat_d[:])
                wvg_s = consts.tile([P, KT, SL], BF)
                nc.sync.dma_start(wvg_s[:], wvg_d[:])
                w3_r = consts.tile([P, HPC, D], BF)
                nc.sync.dma_start(w3_r[:], w3_d[:])
                biasf = consts.tile([1, D], BF)
                nc.sync.dma_start(biasf[:], biasf_d[:])

                ident_bf = consts.tile([P, P], BF)
                from concourse.masks import make_identity
                make_identity(nc, ident_bf)
                onesrow_bf = consts.tile([1, P], BF)
                nc.any.memset(onesrow_bf[:], 1.0)
                ones8_bf = consts.tile([1, NB], BF)
                nc.any.memset(ones8_bf[:], 1.0)
                eps_col = consts.tile([P, 1], F32)
                nc.any.memset(eps_col[:], EPS)

                # ---- persistent state ----
                xbf = res.tile([P, NT, D], BF)        # resident x (bf16)
                musig = res.tile([P, NT, 2], BF)      # [mu, sig] per token
                u_acc = res.tile([P, 2, D], F32)
                zr_acc = res.tile([P, 2, 2, NQ], F32)  # [mh, (r,Z), quarter]

                # ================= PASS 1 =================
                with (
                    tc.tile_pool(name="epool", bufs=1) as ep,
                    tc.tile_pool(name="p1sb", bufs=1) as sb,
                    tc.tile_pool(name="p1ps", bufs=1, space="PSUM") as ps,
                    tc.tile_pool(name="p1pu", bufs=1, space="PSUM") as psu,
                    tc.tile_pool(name="p1pzr", bufs=1, space="PSUM") as pszr,
                ):
                    etil = ep.tile([P, NT, HL], BF)   # etilde per token
                    # stream all of x up front (resident bf16; host pre-cast).
                    # Issue on the ACT HWDGE ring so the SP ring stays
                    # transpose-only (xbar-mode transitions serialize a ring).
                    for ti in range(NT):
                        nc.scalar.dma_start(xbf[:, ti, :],
                                            x_d[ti * P:(ti + 1) * P, :])
                    for q in range(NQ):
                        for lt in range(TPQ):
                            ti = q * TPQ + lt
                            # stats on bf16 x
                            bns = sb.tile([P, 4, 6], F32, tag="bns", bufs=3)
                            for a in range(4):
                                nc.vector.bn_stats(bns[:, a, :],
                                                   xbf[:, ti, a * 512:(a + 1) * 512])
                            mv = sb.tile([P, 2], F32, tag="mv", bufs=3)
                            nc.vector.bn_aggr(mv[:], bns[:])
                            sig = sb.tile([P, 1], F32, tag="sig", bufs=3)
                            nc.scalar.activation(sig[:], mv[:, 1:2], AF.Sqrt,
                                                 bias=eps_col[:])
                            alpha = sb.tile([P, 1], F32, tag="alpha", bufs=3)
                            nc.vector.reciprocal(alpha[:], sig[:])
                            nc.vector.tensor_copy(out=musig[:, ti, 0:1], in_=mv[:, 0:1])
                            nc.vector.tensor_copy(out=musig[:, ti, 1:2], in_=sig[:])
                            # transpose x tile -> [d, tok]
                            xbfT = sb.tile([P, KT, P], BF, tag="xbfT", bufs=3)
                            nc.sync.dma_start_transpose(xbfT[:], xbf[:, ti, :])
                            # mu as a row (PE transpose)
                            mur_ps = ps.tile([1, P], F32, tag="sc", bufs=3,
                                             name=f"mur{ti}")
                            nc.tensor.matmul(mur_ps[:], musig[:, ti, 0:1],
                                             ident_bf[:], start=True, stop=True)
                            murow = sb.tile([1, P], BF, tag="murow", bufs=3)
                            nc.vector.tensor_copy(out=murow[:], in_=mur_ps[:])
                            # scoresT accumulation: rank-1 (-c*mu) then x.qhat
                            sc_ps = ps.tile([P, HL], F32, tag="sc", bufs=3,
                                            name=f"sc{ti}")
                            nc.tensor.matmul(sc_ps[:], murow[:], cneg[:],
                                             start=True, stop=False)
                            for kt in range(KT):
                                nc.tensor.matmul(sc_ps[:], xbfT[:, kt, :],
                                                 qhatT[:, kt, :],
                                                 start=False, stop=(kt == KT - 1))
                            # etilde = alpha * exp(alpha*s)  (bf16)
                            eraw = sb.tile([P, HL], BF, tag="eraw", bufs=2)
                            nc.scalar.activation(eraw[:], sc_ps[:], AF.Exp,
                                                 scale=alpha[:])
                            nc.vector.tensor_scalar_mul(etil[:, ti, :], eraw[:],
                                                        alpha[:])

                        # ---- u / zr sweep for this quarter ----
                        for mh in range(2):
                            psum_u = psu.tile([P, D], F32, tag="u", bufs=1,
                                              name=f"u{q}_{mh}")
                            zr_ps = pszr.tile([P, 2], F32, tag="zr", bufs=1,
                                              name=f"zr{q}_{mh}")
                            for lt in range(TPQ):
                                ti = q * TPQ + lt
                                lhs = etil[:, ti, mh * P:(mh + 1) * P]
                                for nch in range(NCH):
                                    nc.tensor.matmul(
                                        psum_u[:, nch * 512:(nch + 1) * 512],
                                        lhs, xbf[:, ti, nch * 512:(nch + 1) * 512],
                                        start=(lt == 0), stop=(lt == TPQ - 1),
                                        skip_group_check=True)
                                nc.tensor.matmul(zr_ps[:], lhs, musig[:, ti, :],
                                                 start=(lt == 0), stop=(lt == TPQ - 1),
                                                 skip_group_check=True)
                            if q == 0:
                                nc.vector.tensor_copy(out=u_acc[:, mh, :],
                                                      in_=psum_u[:])
                            else:
                                nc.vector.tensor_tensor(u_acc[:, mh, :],
                                                        u_acc[:, mh, :], psum_u[:],
                                                        mybir.AluOpType.add)
                            nc.vector.tensor_copy(out=zr_acc[:, mh, :, q],
                                                  in_=zr_ps[:])

                # ================= STAGE C =================
                with tc.tile_pool(name="c_sb", bufs=1) as csb:
                    # r, Z totals; M' = (u - r)/Z in place
                    zrt = csb.tile([P, 2, 2], F32)
                    nc.vector.tensor_reduce(zrt[:], zr_acc[:], mybir.AxisListType.X,
                                            mybir.AluOpType.add)
                    rzi = csb.tile([P, 2, 1], F32)
                    nc.vector.reciprocal(rzi[:], zrt[:, :, 1:2])
                    for mh in range(2):
                        nc.vector.tensor_scalar(u_acc[:, mh, :], u_acc[:, mh, :],
                                                zrt[:, mh, 0:1], rzi[:, mh, :],
                                                mybir.AluOpType.subtract,
                                                mybir.AluOpType.mult)
                    # mbar = per-head mean [H, D] (bf16)
                    mb_bf = csb.tile([H, D], BF)
                    with tc.tile_pool(name="c_ps_mb", bufs=1, space="PSUM") as cps0:
                        mb_ps = cps0.tile([H, D], F32)
                        for mh in range(2):
                            for nch in range(NCH):
                                nc.tensor.matmul(mb_ps[:, nch * 512:(nch + 1) * 512],
                                                 selmat[:, mh, :],
                                                 u_acc[:, mh, nch * 512:(nch + 1) * 512],
                                                 start=(mh == 0), stop=(mh == 1),
                                                 skip_group_check=True)
                        nc.scalar.copy(out=mb_bf[:], in_=mb_ps[:])
                    nc.sync.dma_start(mb_bounce[:], mb_bf[:])
                    nc.gpsimd.collective_compute(
                        "AllToAll", mybir.AluOpType.bypass, replica_groups=groups,
                        ins=[mb_bounce[:].opt()], outs=[m2_bounce[:].opt()])
                    # m2 rows = (batch b, local head h2); transpose -> [d, (b,h2)]
                    m2T = csb.tile([P, KT, H], BF)
                    nc.sync.dma_start_transpose(m2T[:], m2_bounce[:])
                    # cb[h2] = mb_h @ Wvg_slice block  -> transpose -> [j, b]
                    cbT = csb.tile([P, HPC, NB], BF)
                    with tc.tile_pool(name="c_ps_cb", bufs=1, space="PSUM") as cps1:
                        for h2 in range(HPC):
                            cb_ps = cps1.tile([NB, P], F32, tag="cbps", bufs=2)
                            for kt in range(KT):
                                lhs = m2T[:, kt, :].rearrange("p (b h) -> p h b", h=HPC)
                                nc.tensor.matmul(cb_ps[:], lhs[:, h2, :],
                                                 wvg_s[:, kt, h2 * P:(h2 + 1) * P],
                                                 start=(kt == 0), stop=(kt == KT - 1))
                            cb_sb = csb.tile([NB, P], BF, tag="cbsb", bufs=2)
                            nc.scalar.copy(out=cb_sb[:], in_=cb_ps[:])
                            ct_ps = cps1.tile([P, NB], F32, tag="ctps", bufs=2)
                            nc.tensor.matmul(ct_ps[:], cb_sb[:], ident_bf[:NB, :NB],
                                             start=True, stop=True)
                            nc.scalar.copy(out=cbT[:, h2, :], in_=ct_ps[:])
                    # partial out rows for all batches: po = cb @ W3_rows + biasf/NB
                    with tc.tile_pool(name="c_ps_po", bufs=1, space="PSUM") as cps2:
                        po_ps = cps2.tile([NB, D], F32)
                        for h2 in range(HPC):
                            for nch in range(NCH):
                                nc.tensor.matmul(po_ps[:, nch * 512:(nch + 1) * 512],
                                                 cbT[:, h2, :],
                                                 w3_r[:, h2, nch * 512:(nch + 1) * 512],
                                                 start=(h2 == 0), stop=False,
                                                 skip_group_check=True)
                        for nch in range(NCH):
                            nc.tensor.matmul(po_ps[:, nch * 512:(nch + 1) * 512],
                                             ones8_bf[:],
                                             biasf[:, nch * 512:(nch + 1) * 512],
                                             start=False, stop=(nch == NCH - 1),
                                             skip_group_check=True)
                        for half in range(2):
                            po_sb = csb.tile([NB, D // 2], F32, tag="posb", bufs=1)
                            nc.scalar.copy(out=po_sb[:],
                                           in_=po_ps[:, half * 1024:(half + 1) * 1024])
                            nc.sync.dma_start(
                                po_bounce[:, half * 1024:(half + 1) * 1024], po_sb[:])
                    nc.gpsimd.collective_compute(
                        "ReduceScatter", mybir.AluOpType.add, replica_groups=groups,
                        ins=[po_bounce[:].opt()], outs=[ob_bounce[:].opt()])
                    # broadcast own out row to 128 partitions (bf16)
                    ob_bf = csb.tile([1, D], BF)
                    nc.gpsimd.dma_start(ob_bf[:], ob_bounce[:])  # f32->bf16 cast
                    obb = res.tile([P, D], BF)
                    with tc.tile_pool(name="c_ps_bc", bufs=1, space="PSUM") as cps3:
                        bc_ps = cps3.tile([P, D], F32)
                        for nch in range(NCH):
                            nc.tensor.matmul(bc_ps[:, nch * 512:(nch + 1) * 512],
                                             onesrow_bf[:],
                                             ob_bf[:, nch * 512:(nch + 1) * 512],
                                             start=True, stop=True,
                                             skip_group_check=True)
                        nc.scalar.copy(out=obb[:], in_=bc_ps[:])

                # ================= PASS 2 (residual, no x re-read) =========
                with tc.tile_pool(name="res2", bufs=1) as r2:
                    for ti in range(NT):
                        yt = r2.tile([P, D], F32, tag="yt", bufs=3)
                        eng = nc.vector if ti % 2 == 0 else nc.gpsimd
                        eng.tensor_tensor(yt[:], xbf[:, ti, :], obb[:],
                                          mybir.AluOpType.add)
                        nc.sync.dma_start(y_d[ti * P:(ti + 1) * P, :], yt[:])

    nc.compile()
    return nc


@functools.lru_cache(maxsize=2)
def _built(n_cores: int, S: int):
    return _build(n_cores, S)


def _host_prep(inputs, n_cores: int):
    """Weight folding on host. Returns (global_map, per_core_maps)."""
    NB = n_cores
    HPC = H // NB
    SL = D // NB
    f32 = np.float32
    bf16 = ml_dtypes.bfloat16

    x_all = np.ascontiguousarray(np.asarray(inputs["hidden_states"], dtype=f32))
    g = np.asarray(inputs["ln_g"], dtype=f32)
    b_ln = np.asarray(inputs["ln_b"], dtype=f32)
    lat = np.asarray(inputs["latents"], dtype=f32)
    w_lq = np.asarray(inputs["w_lq"], dtype=f32)
    b_lq = np.asarray(inputs["b_lq"], dtype=f32)
    w_k = np.asarray(inputs["w_k"], dtype=f32)
    w_v = np.asarray(inputs["w_v"], dtype=f32)
    b_v = np.asarray(inputs["b_v"], dtype=f32)
    w_lv = np.asarray(inputs["w_lv"], dtype=f32)
    b_lv = np.asarray(inputs["b_lv"], dtype=f32)
    w_out = np.asarray(inputs["w_out"], dtype=f32)
    b_out = np.asarray(inputs["b_out"], dtype=f32)

    q_full = lat @ w_lq + b_lq                      # [L, D]
    qhatT = np.empty((D, HL), f32)
    for h in range(H):
        qh = q_full[:, HD * h:HD * (h + 1)]          # [L, 128]
        qhatT[:, L * h:L * (h + 1)] = w_k[:, HD * h:HD * (h + 1)] @ qh.T
    qhatT *= g[:, None] * INV_SQRT_HD               # fold 1/sqrt(hd)
    c_vec = qhatT.sum(axis=0)                        # [HL] (already scaled)

    def tile_rows(a):  # [D, N] -> [P, KT, N] with d = t*128 + p
        return np.ascontiguousarray(a.reshape(-1, P, a.shape[-1]).transpose(1, 0, 2))

    qhatT_t = tile_rows(qhatT).astype(bf16)
    cneg = (-c_vec)[None, :].astype(bf16)

    selmat = np.zeros((P, 2, H), f32)
    for mh in range(2):
        for p in range(P):
            selmat[p, mh, (mh * P + p) // L] = 1.0 / L

    wvg = w_v * g[:, None]
    w3 = w_lv @ w_out                                # folded Wlv@Wout [D, D]
    bv_fold = b_v + b_ln @ w_v
    biasf_full = ((bv_fold @ w_lv + b_lv) @ w_out + b_out) / NB

    global_map = {
        "qhatT": qhatT_t, "cneg": cneg, "selmat": selmat,
        "biasf": np.ascontiguousarray(biasf_full[None, :].astype(bf16)),
    }
    per_core = []
    for c in range(NB):
        sl = slice(SL * c, SL * (c + 1))
        wvg_s = tile_rows(wvg[:, sl]).astype(bf16)               # [P, KT, SL]
        w3_rows = np.ascontiguousarray(
            w3[sl, :].reshape(HPC, P, D).transpose(1, 0, 2)).astype(bf16)
        per_core.append({
            "x": np.ascontiguousarray(x_all[c].astype(bf16)),
            "wvg_s": wvg_s, "w3_r": w3_rows,
        })
    return global_map, per_core


def kernel(**inputs) -> np.ndarray:
    NB = 8
    x_all = np.asarray(inputs["hidden_states"])
    B, S, D_ = x_all.shape
    assert D_ == D and B == NB
    nc = _built(NB, S)
    global_map, per_core = _host_prep(inputs, NB)
    in_maps = [{**global_map, **pc} for pc in per_core]
    res = run_bass_kernel_spmd(nc, in_maps, list(range(NB)))
    out = np.stack([res.results[i]["y"] for i in range(NB)], axis=0)
    return out.astype(np.float32)
